# revision 10
# baseline (speedup 1.0000x reference)
"""Trainium2 Bass kernel for nn_AttentionBlock (GroupNorm + 8-head self-attention
+ projection + residual) on x: [16, 512, 32, 32].

Sharding: data-parallel over batch across 8 NeuronCores (2 batch items/core),
no collectives.

Per-core dataflow (per batch item, all matmuls in float32r = full-rate
single-pass reduced-precision fp32):
  1. GroupNorm: per-channel sum / sum-of-squares on DVE, cross-partition group
     reduce + group->channel expansion via tiny PE matmuls against 0/1
     membership matrices, rstd = exp(-0.5*ln(var+eps)) on ACT, apply as
     per-partition x*a+b (DVE tensor_scalar).
  2. qkv: q,k produced in [channel, t] layout (lhsT = w_qkvT chunks); v produced
     directly TRANSPOSED, [t, channel], by using xn chunks as the stationary
     operand - so the attention value-matmul needs no transposes. The attention
     scale 1/sqrt(sqrt(64)) is folded into w_q/w_k host-side. A column of ones
     is appended per head to v^T so the value matmul also emits the softmax
     denominator.
  3. Attention per head: S^T[ts,tq] = k^T q (K=64), exp on ACT straight out of
     PSUM (scores are ~N(0,1): no max subtraction needed), value matmul
     a_un[65, tq] = (v^T|1)^T @ E accumulated over ts chunks; row 64 is the
     denominator. reciprocal_approx_fast on the denominator row, gpsimd
     partition_broadcast, normalize on DVE during PSUM evacuation.
  4. proj + bias + residual fused in one scalar_tensor_tensor per tile.
     b_proj is host-adjusted by w_proj @ b_qkv_v (exact: softmax rows sum to 1).
"""
import math
import sys

sys.path.insert(0, "/opt/trn_rl_repo")

import numpy as np

import concourse.bass as bass  # noqa: F401  (registers types)
import concourse.tile as tile
from concourse import bacc, mybir
from concourse.bass_utils import run_bass_kernel_spmd

AF = mybir.ActivationFunctionType
ALU = mybir.AluOpType
F32 = mybir.dt.float32
F32R = mybir.dt.float32r
BF16 = mybir.dt.bfloat16

B, C, HH, WW = 16, 512, 32, 32
T = HH * WW            # 1024
NH, CH = 8, 64         # heads, head dim
G, CPG = 32, 16        # groupnorm groups, channels per group
EPS = 1e-5
NCORES = 8
BPC = B // NCORES      # 2 batch items per core
P = 128
NCC = C // P           # 4 channel chunks
NTC = T // P           # 8 t chunks
NN = T // 512          # 2 n-chunks of 512
DBG = False            # extra intermediate dumps (debugging only)
QKV_BF16 = True        # bf16 operands for qkv/proj matmuls (else float32r)
PACK_S = True          # 2-head tile_position packing for the K=64 score matmuls


def _body(ctx, tc, d):
    nc = tc.nc
    sync = nc.sync

    consts = ctx.enter_context(tc.tile_pool(name="consts", bufs=1))
    xp = ctx.enter_context(tc.tile_pool(name="xp", bufs=2))
    xnp = ctx.enter_context(tc.tile_pool(name="xnp", bufs=1))
    qkp = ctx.enter_context(tc.tile_pool(name="qkp", bufs=1))
    vtp = ctx.enter_context(tc.tile_pool(name="vtp", bufs=1))
    apl = ctx.enter_context(tc.tile_pool(name="apl", bufs=1))
    ep = ctx.enter_context(tc.tile_pool(name="ep", bufs=4))
    smp = ctx.enter_context(tc.tile_pool(name="smp", bufs=2))
    rp = ctx.enter_context(tc.tile_pool(name="rp", bufs=2))
    opl = ctx.enter_context(tc.tile_pool(name="opl", bufs=4))
    ps = ctx.enter_context(tc.tile_pool(name="ps", bufs=2, space="PSUM"))

    WDT = BF16 if QKV_BF16 else F32R
    # ---- constants ----
    wqkvT = consts.tile([P, NCC, 3 * C], WDT)   # [128, 4, 1536]
    sync.dma_start(wqkvT[:], d["wqkvT"].rearrange("(cc p) o -> p cc o", p=P))
    wprojT = consts.tile([P, NCC, C], WDT)      # [128, 4, 512]
    sync.dma_start(wprojT[:], d["wprojT"].rearrange("(cc p) o -> p cc o", p=P))
    aux = consts.tile([P, 20], F32)              # bqk[0:8] bproj[8:12] gns[12:16] gnb[16:20]
    sync.dma_start(aux[:], d["aux"][:])
    gmats = consts.tile([P, NCC, G], F32)        # channel -> group membership (per chunk)
    sync.dma_start(gmats[:], d["gmats"][:])
    ematT = consts.tile([G, NCC, P], F32)        # group -> channel expansion (per chunk)
    sync.dma_start(ematT[:], d["ematT"][:])
    ones8 = consts.tile([P, NH], BF16)
    sync.dma_start(ones8[:], d["ones"][:])

    psidx = [0]

    def acc_tile(name):
        t_ = ps.tile([P, 512], F32, tag=("big", "acc")[psidx[0] % 2], name=name)
        psidx[0] += 1
        return t_

    for bi in range(BPC):
        # ---- load x ----
        xb = []
        for c in range(NCC):
            xt = xp.tile([P, T], F32, tag=f"x{c}", name=f"x_{bi}_{c}")
            sync.dma_start(xt[:], d["x"][bi, c * P:(c + 1) * P, :])
            xb.append(xt)

        # ---- GroupNorm stats ----
        s12 = smp.tile([P, NCC, 2], F32, tag="s12", name=f"s12_{bi}")
        xnb = []
        for c in range(NCC):
            xnt = xnp.tile([P, T], WDT, tag=f"xn{c}", name=f"xn_{bi}_{c}")
            xnb.append(xnt)
            nc.vector.reduce_sum(s12[:, c, 0:1], xb[c][:], axis=mybir.AxisListType.X)
            # x^2 into xn scratch (overwritten later), accumulate sum(x^2)
            nc.vector.scalar_tensor_tensor(
                xnt[:], xb[c][:], 1.0, xb[c][:],
                op0=ALU.mult, op1=ALU.mult, accum_out=s12[:, c, 1:2])

        gsum = ps.tile([G, 2], F32, tag="acc", name=f"gsum_{bi}")
        for c in range(NCC):
            nc.tensor.matmul(gsum[:], gmats[:, c, :], s12[:, c, :],
                             start=(c == 0), stop=(c == NCC - 1))

        ms = smp.tile([G, 4], F32, tag="ms", name=f"ms_{bi}")  # mu, msq, var, mu^2
        nc.vector.tensor_scalar_mul(ms[:, 0:2], gsum[:], 1.0 / (CPG * T))
        nc.vector.tensor_mul(ms[:, 3:4], ms[:, 0:1], ms[:, 0:1])
        # var + eps = (msq + eps) - mu^2
        nc.vector.scalar_tensor_tensor(ms[:, 2:3], ms[:, 1:2], EPS, ms[:, 3:4],
                                       op0=ALU.add, op1=ALU.subtract)
        musd = smp.tile([G, 2], F32, tag="musd", name=f"musd_{bi}")  # mu, rstd
        nc.vector.tensor_copy(musd[:, 0:1], ms[:, 0:1])
        lnv = smp.tile([G, 1], F32, tag="lnv", name=f"lnv_{bi}")
        nc.scalar.activation(lnv[:], ms[:, 2:3], AF.Ln)
        nc.scalar.activation(musd[:, 1:2], lnv[:], AF.Exp, scale=-0.5)

        for c in range(NCC):
            chan = ps.tile([P, 2], F32, tag="big", name=f"chan_{bi}_{c}")
            nc.tensor.matmul(chan[:], ematT[:, c, :], musd[:], start=True, stop=True)
            ac = smp.tile([P, 3], F32, tag=f"aff{c}", name=f"aff_{bi}_{c}")  # a, -a, b
            nc.vector.tensor_mul(ac[:, 0:1], aux[:, 12 + c:13 + c], chan[:, 1:2])
            nc.vector.tensor_scalar_mul(ac[:, 1:2], ac[:, 0:1], -1.0)
            nc.vector.scalar_tensor_tensor(
                ac[:, 2:3], chan[:, 0:1], ac[:, 1:2], aux[:, 16 + c:17 + c],
                op0=ALU.mult, op1=ALU.add)
            nc.vector.tensor_scalar(
                out=xnb[c][:], in0=xb[c][:], scalar1=ac[:, 0:1], scalar2=ac[:, 2:3],
                op0=ALU.mult, op1=ALU.add)
            if DBG and bi == 0:
                sync.dma_start(d["dbg_xn"][c * P:(c + 1) * P, :], xnb[c][:])

        # ---- qkv ----
        qt = [qkp.tile([P, T], BF16, tag=f"q{oc}", name=f"q_{bi}_{oc}") for oc in range(NCC)]
        kt = [qkp.tile([P, T], BF16, tag=f"k{oc}", name=f"k_{bi}_{oc}") for oc in range(NCC)]
        for dst, base, boff in ((qt, 0, 0), (kt, C, 4)):
            for oc in range(NCC):
                accs = [acc_tile(f"qk_{bi}_{base}_{oc}_{n}") for n in range(NN)]
                for kc in range(NCC):
                    for n in range(NN):
                        nc.tensor.matmul(
                            accs[n][:],
                            wqkvT[:, kc, base + oc * P:base + (oc + 1) * P],
                            xnb[kc][:, n * 512:(n + 1) * 512],
                            start=(kc == 0), stop=(kc == NCC - 1))
                for n in range(NN):
                    nc.vector.tensor_scalar_add(
                        dst[oc][:, n * 512:(n + 1) * 512], accs[n][:],
                        aux[:, boff + oc:boff + oc + 1])
        if DBG and bi == 0:
            for oc in range(NCC):
                sync.dma_start(d["dbg_q"][oc * P:(oc + 1) * P, :], qt[oc][:])
                sync.dma_start(d["dbg_k"][oc * P:(oc + 1) * P, :], kt[oc][:])
        vt = []
        for m in range(NTC):
            vtt = vtp.tile([P, NH, CH + 1], BF16, tag=f"vt{m}", name=f"vt_{bi}_{m}")
            nc.vector.tensor_copy(vtt[:, :, CH:CH + 1],
                                  ones8[:].rearrange("p (h o) -> p h o", o=1))
            acc = acc_tile(f"v_{bi}_{m}")
            for kc in range(NCC):
                nc.tensor.matmul(acc[:], xnb[kc][:, m * P:(m + 1) * P],
                                 wqkvT[:, kc, 2 * C:3 * C],
                                 start=(kc == 0), stop=(kc == NCC - 1))
            nc.vector.tensor_copy(vtt[:, :, 0:CH],
                                  acc[:].rearrange("p (h c) -> p h c", c=CH))
            vt.append(vtt)
            pass

        # ---- attention ----
        at = [apl.tile([P, T], WDT, tag=f"a{cc}", name=f"a_{bi}_{cc}") for cc in range(NCC)]
        for hp in range(NH // 2):
            heads = (2 * hp, 2 * hp + 1)
            a_accs = {}
            for h in heads:
                a_accs[h] = ps.tile([P, T], F32, tag="acc", name=f"aacc_{bi}_{h}")
            for m in range(NTC):
                sps = {}
                for h in heads:
                    po = (h % 2) * CH
                    qh = qt[hp][po:po + CH, :]
                    kh = kt[hp][po:po + CH, :]
                    spst = ps.tile([P, T], F32, tag="big", name=f"s_{bi}_{h}_{m}")
                    sps[h] = spst
                    for n in range(NN):
                        nc.tensor.matmul(spst[:, n * 512:(n + 1) * 512],
                                         kh[:, m * P:(m + 1) * P],
                                         qh[:, n * 512:(n + 1) * 512],
                                         start=True, stop=True,
                                         tile_position=((h % 2) * CH, 0) if PACK_S else None)
                for h in heads:
                    et = ep.tile([P, T], BF16, tag="e", name=f"e_{bi}_{h}_{m}")
                    nc.scalar.activation(et[:], sps[h][:], AF.Exp)
                    for n in range(NN):
                        nc.tensor.matmul(a_accs[h][0:CH + 1, n * 512:(n + 1) * 512],
                                         vt[m][:, h, :],
                                         et[:, n * 512:(n + 1) * 512],
                                         start=(m == 0), stop=(m == NTC - 1))
            for h in heads:
                po = (h % 2) * CH
                den0 = rp.tile([1, T], F32, tag="den0", name=f"dn_{bi}_{h}")
                nc.vector.tensor_copy(den0[:], a_accs[h][CH:CH + 1, :])
                rrow = rp.tile([1, T], F32, tag="rrow", name=f"rr_{bi}_{h}")
                nc.vector.reciprocal_approx_fast(rrow[:], den0[:])
                rb = rp.tile([CH, T], F32, tag="rb", name=f"rb_{bi}_{h}")
                nc.gpsimd.partition_broadcast(rb[:], rrow[:])
                nc.vector.tensor_mul(at[hp][po:po + CH, :], a_accs[h][0:CH, :], rb[:])

        if DBG and bi == 0:
            for cc in range(NCC):
                sync.dma_start(d["dbg_a"][cc * P:(cc + 1) * P, :], at[cc][:])
        # ---- proj + bias + residual ----
        for oc in range(NCC):
            accs = [acc_tile(f"p_{bi}_{oc}_{n}") for n in range(NN)]
            for kc in range(NCC):
                for n in range(NN):
                    nc.tensor.matmul(accs[n][:],
                                     wprojT[:, kc, oc * P:(oc + 1) * P],
                                     at[kc][:, n * 512:(n + 1) * 512],
                                     start=(kc == 0), stop=(kc == NCC - 1))
            for n in range(NN):
                ot = opl.tile([P, 512], F32, tag="o", name=f"o_{bi}_{oc}_{n}")
                nc.vector.scalar_tensor_tensor(
                    ot[:], accs[n][:], aux[:, 8 + oc:9 + oc],
                    xb[oc][:, n * 512:(n + 1) * 512],
                    op0=ALU.add, op1=ALU.add)
                sync.dma_start(d["out"][bi, oc * P:(oc + 1) * P, n * 512:(n + 1) * 512],
                               ot[:])


def build():
    from contextlib import ExitStack

    nc = bacc.Bacc("TRN2", target_bir_lowering=False, debug=False,
                   num_devices=NCORES)
    d = {
        "x": nc.dram_tensor("x", [BPC, C, T], F32, kind="ExternalInput").ap(),
        "wqkvT": nc.dram_tensor("wqkvT", [C, 3 * C], BF16 if QKV_BF16 else F32R, kind="ExternalInput").ap(),
        "wprojT": nc.dram_tensor("wprojT", [C, C], BF16 if QKV_BF16 else F32R, kind="ExternalInput").ap(),
        "aux": nc.dram_tensor("aux", [P, 20], F32, kind="ExternalInput").ap(),
        "gmats": nc.dram_tensor("gmats", [P, NCC, G], F32, kind="ExternalInput").ap(),
        "ematT": nc.dram_tensor("ematT", [G, NCC, P], F32, kind="ExternalInput").ap(),
        "ones": nc.dram_tensor("ones", [P, NH], BF16, kind="ExternalInput").ap(),
        "out": nc.dram_tensor("out", [BPC, C, T], F32, kind="ExternalOutput").ap(),
    }
    if DBG:
        for nm, shp in (("dbg_xn", [C, T]), ("dbg_q", [C, T]), ("dbg_k", [C, T]),
                        ("dbg_vt", [T, NH * (CH + 1)]), ("dbg_a", [C, T]),
                        ("dbg_rb", [CH, T])):
            d[nm] = nc.dram_tensor(nm, shp, F32R, kind="ExternalOutput").ap()
    with tile.TileContext(nc) as tc:
        with ExitStack() as ctx:
            _body(ctx, tc, d)
    nc.compile()
    return nc


_CACHE = {}


def prep_inputs(x, gn_scale, gn_bias, w_qkv, b_qkv, w_proj, b_proj):
    x = np.ascontiguousarray(np.asarray(x, np.float32).reshape(B, C, T))
    gn_scale = np.asarray(gn_scale, np.float32)
    gn_bias = np.asarray(gn_bias, np.float32)
    w_qkv = np.asarray(w_qkv, np.float32)
    b_qkv = np.asarray(b_qkv, np.float32)
    w_proj = np.asarray(w_proj, np.float32)
    b_proj = np.asarray(b_proj, np.float32)

    s = 1.0 / math.sqrt(math.sqrt(CH))
    wqkvT = w_qkv.T.copy()                      # [512, 1536]
    wqkvT[:, :2 * C] *= s                       # fold attention scale into q,k
    wprojT = w_proj.T.copy()                    # [512, 512]

    bqk = (b_qkv[:2 * C] * s).reshape(2 * NCC, P).T          # [128, 8]
    bproj_eff = (b_proj + w_proj @ b_qkv[2 * C:]).reshape(NCC, P).T  # [128, 4]
    gns = gn_scale.reshape(NCC, P).T
    gnb = gn_bias.reshape(NCC, P).T
    aux = np.ascontiguousarray(
        np.concatenate([bqk, bproj_eff, gns, gnb], axis=1), np.float32)

    p = np.arange(P)
    gmats = np.zeros((P, NCC, G), np.float32)
    ematT = np.zeros((G, NCC, P), np.float32)
    for c in range(NCC):
        gmats[p, c, 8 * c + p // CPG] = 1.0
        ematT[8 * c + p // CPG, c, p] = 1.0

    import ml_dtypes
    wdt = ml_dtypes.bfloat16 if QKV_BF16 else np.float32
    shared = {"wqkvT": np.ascontiguousarray(wqkvT).astype(wdt),
              "wprojT": np.ascontiguousarray(wprojT).astype(wdt),
              "aux": aux, "gmats": gmats, "ematT": ematT,
              "ones": np.ones((P, NH), ml_dtypes.bfloat16)}
    in_maps = []
    for ci in range(NCORES):
        m = dict(shared)
        m["x"] = np.ascontiguousarray(x[BPC * ci:BPC * (ci + 1)])
        in_maps.append(m)
    return in_maps


def run(inputs, trace=False, tmpdir=None):
    if "nc" not in _CACHE:
        _CACHE["nc"] = build()
    nc = _CACHE["nc"]
    in_maps = prep_inputs(**inputs)
    kwargs = {}
    if trace:
        kwargs["trace"] = True
    if tmpdir:
        kwargs["tmpdir"] = tmpdir
    res = run_bass_kernel_spmd(nc, in_maps, core_ids=list(range(NCORES)), **kwargs)
    out = np.concatenate([r["out"] for r in res.results], axis=0)
    return out.reshape(B, C, HH, WW), res


def kernel(**inputs):
    return run(inputs)[0]


# revision 11
# speedup vs baseline: 1.1023x; 1.1023x over previous
"""Trainium2 Bass kernel for nn_AttentionBlock (GroupNorm + 8-head self-attention
+ projection + residual) on x: [16, 512, 32, 32].

Sharding: data-parallel over batch across 8 NeuronCores (2 batch items/core),
no collectives.

Per-core dataflow (per batch item, all matmuls in float32r = full-rate
single-pass reduced-precision fp32):
  1. GroupNorm: per-channel sum / sum-of-squares on DVE, cross-partition group
     reduce + group->channel expansion via tiny PE matmuls against 0/1
     membership matrices, rstd = exp(-0.5*ln(var+eps)) on ACT, apply as
     per-partition x*a+b (DVE tensor_scalar).
  2. qkv: q,k produced in [channel, t] layout (lhsT = w_qkvT chunks); v produced
     directly TRANSPOSED, [t, channel], by using xn chunks as the stationary
     operand - so the attention value-matmul needs no transposes. The attention
     scale 1/sqrt(sqrt(64)) is folded into w_q/w_k host-side. A column of ones
     is appended per head to v^T so the value matmul also emits the softmax
     denominator.
  3. Attention per head: S^T[ts,tq] = k^T q (K=64), exp on ACT straight out of
     PSUM (scores are ~N(0,1): no max subtraction needed), value matmul
     a_un[65, tq] = (v^T|1)^T @ E accumulated over ts chunks; row 64 is the
     denominator. reciprocal_approx_fast on the denominator row, gpsimd
     partition_broadcast, normalize on DVE during PSUM evacuation.
  4. proj + bias + residual fused in one scalar_tensor_tensor per tile.
     b_proj is host-adjusted by w_proj @ b_qkv_v (exact: softmax rows sum to 1).
"""
import math
import sys

sys.path.insert(0, "/opt/trn_rl_repo")

import numpy as np

import concourse.bass as bass  # noqa: F401  (registers types)
import concourse.tile as tile
from concourse import bacc, mybir
from concourse.bass_utils import run_bass_kernel_spmd

AF = mybir.ActivationFunctionType
ALU = mybir.AluOpType
F32 = mybir.dt.float32
F32R = mybir.dt.float32r
BF16 = mybir.dt.bfloat16

B, C, HH, WW = 16, 512, 32, 32
T = HH * WW            # 1024
NH, CH = 8, 64         # heads, head dim
G, CPG = 32, 16        # groupnorm groups, channels per group
EPS = 1e-5
NCORES = 8
BPC = B // NCORES      # 2 batch items per core
P = 128
NCC = C // P           # 4 channel chunks
NTC = T // P           # 8 t chunks
NN = T // 512          # 2 n-chunks of 512
DBG = False            # extra intermediate dumps (debugging only)
QKV_BF16 = True        # bf16 operands for qkv/proj matmuls (else float32r)
PACK_S = True          # 2-head tile_position packing for the K=64 score matmuls


def _body(ctx, tc, d):
    nc = tc.nc
    sync = nc.sync

    consts = ctx.enter_context(tc.tile_pool(name="consts", bufs=1))
    xp = ctx.enter_context(tc.tile_pool(name="xp", bufs=2))
    xnp = ctx.enter_context(tc.tile_pool(name="xnp", bufs=1))
    qkp = ctx.enter_context(tc.tile_pool(name="qkp", bufs=1))
    vtp = ctx.enter_context(tc.tile_pool(name="vtp", bufs=1))
    apl = ctx.enter_context(tc.tile_pool(name="apl", bufs=1))
    ep = ctx.enter_context(tc.tile_pool(name="ep", bufs=4))
    smp = ctx.enter_context(tc.tile_pool(name="smp", bufs=2))
    rp = ctx.enter_context(tc.tile_pool(name="rp", bufs=2))
    opl = ctx.enter_context(tc.tile_pool(name="opl", bufs=4))
    ps = ctx.enter_context(tc.tile_pool(name="ps", bufs=2, space="PSUM"))

    WDT = BF16 if QKV_BF16 else F32R
    # ---- constants ----
    wqkvT = consts.tile([P, NCC, 3 * C], WDT)   # [128, 4, 1536]
    sync.dma_start(wqkvT[:], d["wqkvT"].rearrange("(cc p) o -> p cc o", p=P))
    wprojT = consts.tile([P, NCC, C], WDT)      # [128, 4, 512]
    sync.dma_start(wprojT[:], d["wprojT"].rearrange("(cc p) o -> p cc o", p=P))
    aux = consts.tile([P, 20], F32)              # bqk[0:8] bproj[8:12] gns[12:16] gnb[16:20]
    sync.dma_start(aux[:], d["aux"][:])
    gmats = consts.tile([P, NCC, G], F32)        # channel -> group membership (per chunk)
    sync.dma_start(gmats[:], d["gmats"][:])
    ematT = consts.tile([G, NCC, P], F32)        # group -> channel expansion (per chunk)
    sync.dma_start(ematT[:], d["ematT"][:])
    ones8 = consts.tile([P, NH], BF16)
    sync.dma_start(ones8[:], d["ones"][:])

    psidx = [0]

    def acc_tile(name):
        t_ = ps.tile([P, 512], F32, tag=("big", "acc")[psidx[0] % 2], name=name)
        psidx[0] += 1
        return t_

    for bi in range(BPC):
        # ---- load x ----
        xb = []
        for c in range(NCC):
            xt = xp.tile([P, T], F32, tag=f"x{c}", name=f"x_{bi}_{c}")
            sync.dma_start(xt[:], d["x"][bi, c * P:(c + 1) * P, :])
            xb.append(xt)

        # ---- GroupNorm stats ----
        s12 = smp.tile([P, NCC, 2], F32, tag="s12", name=f"s12_{bi}")
        xnb = []
        for c in range(NCC):
            xnt = xnp.tile([P, T], WDT, tag=f"xn{c}", name=f"xn_{bi}_{c}")
            xnb.append(xnt)
            nc.vector.reduce_sum(s12[:, c, 0:1], xb[c][:], axis=mybir.AxisListType.X)
            # x^2 into xn scratch (overwritten later), accumulate sum(x^2)
            nc.vector.scalar_tensor_tensor(
                xnt[:], xb[c][:], 1.0, xb[c][:],
                op0=ALU.mult, op1=ALU.mult, accum_out=s12[:, c, 1:2])

        gsum = ps.tile([G, 2], F32, tag="acc", name=f"gsum_{bi}")
        for c in range(NCC):
            nc.tensor.matmul(gsum[:], gmats[:, c, :], s12[:, c, :],
                             start=(c == 0), stop=(c == NCC - 1))

        ms = smp.tile([G, 4], F32, tag="ms", name=f"ms_{bi}")  # mu, msq, var, mu^2
        nc.vector.tensor_scalar_mul(ms[:, 0:2], gsum[:], 1.0 / (CPG * T))
        nc.vector.tensor_mul(ms[:, 3:4], ms[:, 0:1], ms[:, 0:1])
        # var + eps = (msq + eps) - mu^2
        nc.vector.scalar_tensor_tensor(ms[:, 2:3], ms[:, 1:2], EPS, ms[:, 3:4],
                                       op0=ALU.add, op1=ALU.subtract)
        musd = smp.tile([G, 2], F32, tag="musd", name=f"musd_{bi}")  # mu, rstd
        nc.vector.tensor_copy(musd[:, 0:1], ms[:, 0:1])
        lnv = smp.tile([G, 1], F32, tag="lnv", name=f"lnv_{bi}")
        nc.scalar.activation(lnv[:], ms[:, 2:3], AF.Ln)
        nc.scalar.activation(musd[:, 1:2], lnv[:], AF.Exp, scale=-0.5)

        for c in range(NCC):
            chan = ps.tile([P, 2], F32, tag="big", name=f"chan_{bi}_{c}")
            nc.tensor.matmul(chan[:], ematT[:, c, :], musd[:], start=True, stop=True)
            ac = smp.tile([P, 3], F32, tag=f"aff{c}", name=f"aff_{bi}_{c}")  # a, -a, b
            nc.vector.tensor_mul(ac[:, 0:1], aux[:, 12 + c:13 + c], chan[:, 1:2])
            nc.vector.tensor_scalar_mul(ac[:, 1:2], ac[:, 0:1], -1.0)
            nc.vector.scalar_tensor_tensor(
                ac[:, 2:3], chan[:, 0:1], ac[:, 1:2], aux[:, 16 + c:17 + c],
                op0=ALU.mult, op1=ALU.add)
            nc.vector.tensor_scalar(
                out=xnb[c][:], in0=xb[c][:], scalar1=ac[:, 0:1], scalar2=ac[:, 2:3],
                op0=ALU.mult, op1=ALU.add)
            if DBG and bi == 0:
                sync.dma_start(d["dbg_xn"][c * P:(c + 1) * P, :], xnb[c][:])

        # ---- qkv ----
        qt = [qkp.tile([P, T], BF16, tag=f"q{oc}", name=f"q_{bi}_{oc}") for oc in range(NCC)]
        kt = [qkp.tile([P, T], BF16, tag=f"k{oc}", name=f"k_{bi}_{oc}") for oc in range(NCC)]
        for dst, base, boff in ((qt, 0, 0), (kt, C, 4)):
            for oc in range(NCC):
                accs = [acc_tile(f"qk_{bi}_{base}_{oc}_{n}") for n in range(NN)]
                for kc in range(NCC):
                    for n in range(NN):
                        nc.tensor.matmul(
                            accs[n][:],
                            wqkvT[:, kc, base + oc * P:base + (oc + 1) * P],
                            xnb[kc][:, n * 512:(n + 1) * 512],
                            start=(kc == 0), stop=(kc == NCC - 1))
                for n in range(NN):
                    nc.vector.tensor_scalar_add(
                        dst[oc][:, n * 512:(n + 1) * 512], accs[n][:],
                        aux[:, boff + oc:boff + oc + 1])
        if DBG and bi == 0:
            for oc in range(NCC):
                sync.dma_start(d["dbg_q"][oc * P:(oc + 1) * P, :], qt[oc][:])
                sync.dma_start(d["dbg_k"][oc * P:(oc + 1) * P, :], kt[oc][:])
        vt = []
        for m in range(NTC):
            vtt = vtp.tile([P, NH, CH + 1], BF16, tag=f"vt{m}", name=f"vt_{bi}_{m}")
            nc.vector.tensor_copy(vtt[:, :, CH:CH + 1],
                                  ones8[:].rearrange("p (h o) -> p h o", o=1))
            acc = acc_tile(f"v_{bi}_{m}")
            for kc in range(NCC):
                nc.tensor.matmul(acc[:], xnb[kc][:, m * P:(m + 1) * P],
                                 wqkvT[:, kc, 2 * C:3 * C],
                                 start=(kc == 0), stop=(kc == NCC - 1))
            nc.vector.tensor_copy(vtt[:, :, 0:CH],
                                  acc[:].rearrange("p (h c) -> p h c", c=CH))
            vt.append(vtt)
            pass

        # ---- attention ----
        at = [apl.tile([P, T], WDT, tag=f"a{cc}", name=f"a_{bi}_{cc}") for cc in range(NCC)]
        for h in range(NH):
            po = (h % 2) * CH
            qh = qt[h // 2][po:po + CH, :]
            kh = kt[h // 2][po:po + CH, :]
            a_acc = ps.tile([P, T], F32, tag="acc", name=f"aacc_{bi}_{h}")
            for m in range(NTC):
                sps = ps.tile([P, T], F32, tag="big", name=f"s_{bi}_{h}_{m}")
                for n in range(NN):
                    nc.tensor.matmul(sps[:, n * 512:(n + 1) * 512],
                                     kh[:, m * P:(m + 1) * P],
                                     qh[:, n * 512:(n + 1) * 512],
                                     start=True, stop=True)
                et = ep.tile([P, T], BF16, tag="e", name=f"e_{bi}_{h}_{m}")
                nc.scalar.activation(et[:], sps[:], AF.Exp)
                for n in range(NN):
                    nc.tensor.matmul(a_acc[0:CH + 1, n * 512:(n + 1) * 512],
                                     vt[m][:, h, :],
                                     et[:, n * 512:(n + 1) * 512],
                                     start=(m == 0), stop=(m == NTC - 1))
            den0 = rp.tile([1, T], F32, tag="den0", name=f"dn_{bi}_{h}")
            nc.vector.tensor_copy(den0[:], a_acc[CH:CH + 1, :])
            rrow = rp.tile([1, T], F32, tag="rrow", name=f"rr_{bi}_{h}")
            nc.vector.reciprocal_approx_fast(rrow[:], den0[:])
            rb = rp.tile([CH, T], F32, tag="rb", name=f"rb_{bi}_{h}")
            nc.gpsimd.partition_broadcast(rb[:], rrow[:])
            nc.vector.tensor_mul(at[h // 2][po:po + CH, :], a_acc[0:CH, :], rb[:])

        if DBG and bi == 0:
            for cc in range(NCC):
                sync.dma_start(d["dbg_a"][cc * P:(cc + 1) * P, :], at[cc][:])
        # ---- proj + bias + residual ----
        for oc in range(NCC):
            accs = [acc_tile(f"p_{bi}_{oc}_{n}") for n in range(NN)]
            for kc in range(NCC):
                for n in range(NN):
                    nc.tensor.matmul(accs[n][:],
                                     wprojT[:, kc, oc * P:(oc + 1) * P],
                                     at[kc][:, n * 512:(n + 1) * 512],
                                     start=(kc == 0), stop=(kc == NCC - 1))
            for n in range(NN):
                ot = opl.tile([P, 512], F32, tag="o", name=f"o_{bi}_{oc}_{n}")
                nc.vector.scalar_tensor_tensor(
                    ot[:], accs[n][:], aux[:, 8 + oc:9 + oc],
                    xb[oc][:, n * 512:(n + 1) * 512],
                    op0=ALU.add, op1=ALU.add)
                sync.dma_start(d["out"][bi, oc * P:(oc + 1) * P, n * 512:(n + 1) * 512],
                               ot[:])


def build():
    from contextlib import ExitStack

    nc = bacc.Bacc("TRN2", target_bir_lowering=False, debug=False,
                   num_devices=NCORES)
    d = {
        "x": nc.dram_tensor("x", [BPC, C, T], F32, kind="ExternalInput").ap(),
        "wqkvT": nc.dram_tensor("wqkvT", [C, 3 * C], BF16 if QKV_BF16 else F32R, kind="ExternalInput").ap(),
        "wprojT": nc.dram_tensor("wprojT", [C, C], BF16 if QKV_BF16 else F32R, kind="ExternalInput").ap(),
        "aux": nc.dram_tensor("aux", [P, 20], F32, kind="ExternalInput").ap(),
        "gmats": nc.dram_tensor("gmats", [P, NCC, G], F32, kind="ExternalInput").ap(),
        "ematT": nc.dram_tensor("ematT", [G, NCC, P], F32, kind="ExternalInput").ap(),
        "ones": nc.dram_tensor("ones", [P, NH], BF16, kind="ExternalInput").ap(),
        "out": nc.dram_tensor("out", [BPC, C, T], F32, kind="ExternalOutput").ap(),
    }
    if DBG:
        for nm, shp in (("dbg_xn", [C, T]), ("dbg_q", [C, T]), ("dbg_k", [C, T]),
                        ("dbg_vt", [T, NH * (CH + 1)]), ("dbg_a", [C, T]),
                        ("dbg_rb", [CH, T])):
            d[nm] = nc.dram_tensor(nm, shp, F32R, kind="ExternalOutput").ap()
    with tile.TileContext(nc) as tc:
        with ExitStack() as ctx:
            _body(ctx, tc, d)
    nc.compile()
    return nc


_CACHE = {}


def prep_inputs(x, gn_scale, gn_bias, w_qkv, b_qkv, w_proj, b_proj):
    x = np.ascontiguousarray(np.asarray(x, np.float32).reshape(B, C, T))
    gn_scale = np.asarray(gn_scale, np.float32)
    gn_bias = np.asarray(gn_bias, np.float32)
    w_qkv = np.asarray(w_qkv, np.float32)
    b_qkv = np.asarray(b_qkv, np.float32)
    w_proj = np.asarray(w_proj, np.float32)
    b_proj = np.asarray(b_proj, np.float32)

    s = 1.0 / math.sqrt(math.sqrt(CH))
    wqkvT = w_qkv.T.copy()                      # [512, 1536]
    wqkvT[:, :2 * C] *= s                       # fold attention scale into q,k
    wprojT = w_proj.T.copy()                    # [512, 512]

    bqk = (b_qkv[:2 * C] * s).reshape(2 * NCC, P).T          # [128, 8]
    bproj_eff = (b_proj + w_proj @ b_qkv[2 * C:]).reshape(NCC, P).T  # [128, 4]
    gns = gn_scale.reshape(NCC, P).T
    gnb = gn_bias.reshape(NCC, P).T
    aux = np.ascontiguousarray(
        np.concatenate([bqk, bproj_eff, gns, gnb], axis=1), np.float32)

    p = np.arange(P)
    gmats = np.zeros((P, NCC, G), np.float32)
    ematT = np.zeros((G, NCC, P), np.float32)
    for c in range(NCC):
        gmats[p, c, 8 * c + p // CPG] = 1.0
        ematT[8 * c + p // CPG, c, p] = 1.0

    import ml_dtypes
    wdt = ml_dtypes.bfloat16 if QKV_BF16 else np.float32
    shared = {"wqkvT": np.ascontiguousarray(wqkvT).astype(wdt),
              "wprojT": np.ascontiguousarray(wprojT).astype(wdt),
              "aux": aux, "gmats": gmats, "ematT": ematT,
              "ones": np.ones((P, NH), ml_dtypes.bfloat16)}
    in_maps = []
    for ci in range(NCORES):
        m = dict(shared)
        m["x"] = np.ascontiguousarray(x[BPC * ci:BPC * (ci + 1)])
        in_maps.append(m)
    return in_maps


def run(inputs, trace=False, tmpdir=None):
    if "nc" not in _CACHE:
        _CACHE["nc"] = build()
    nc = _CACHE["nc"]
    in_maps = prep_inputs(**inputs)
    kwargs = {}
    if trace:
        kwargs["trace"] = True
    if tmpdir:
        kwargs["tmpdir"] = tmpdir
    res = run_bass_kernel_spmd(nc, in_maps, core_ids=list(range(NCORES)), **kwargs)
    out = np.concatenate([r["out"] for r in res.results], axis=0)
    return out.reshape(B, C, HH, WW), res


def kernel(**inputs):
    return run(inputs)[0]
